# revision 12
# baseline (speedup 1.0000x reference)
"""Trainium2 Bass kernel for nn_EntityTable (GRU entity scan over routed projections).

Sharding: data-parallel over batch B=16 -> 2 batches per core (8 cores).
Host pre-transposes h_seq to (D, b, t) per core and all weights, so the
device never transposes. Phase 1 computes hpT = Wi @ h^T and the routing
softmax (stream-major), staging w in (t,b,n) order. Phase 2 runs the
sequential GRU with gx contributions prefilled into PSUM by PE matmuls so
each step is only: 2 matmuls, 1 sigmoid (r,z fused + folded biases),
1 fused (p+b)*r, 1 add, 1 tanh (folded bias), and a 3-op blend with two
ops off the critical path. Output is written (d,b,t,n) and transposed on
host.
"""

import numpy as np
from contextlib import ExitStack

import concourse.bass as bass
import concourse.bacc as bacc
import concourse.tile as tile
from concourse import mybir
from concourse.bass_utils import run_bass_kernel_spmd

F32 = mybir.dt.float32
AF = mybir.ActivationFunctionType
OP = mybir.AluOpType

B, T, D = 16, 2048, 1024
NE, DE = 8, 64
NCORES = 8
BL = B // NCORES          # 2 batches per core
NS = BL * NE              # 16 streams per core, col = b*8 + n
TBL = BL * T              # 4096 (b,t) columns per core


def _expand(ap, dims):
    """Replace the free dims of a sliced AP with an explicit dim list."""
    return bass.AP(tensor=ap.tensor, offset=ap.offset, ap=[list(ap.ap[0])] + dims)


def build_program(T_steps=T, chunk=64, granule=32):
    """Build the SPMD Bass program (same program on all 8 cores)."""
    assert T_steps % chunk == 0 and chunk % granule == 0
    n_iters = T_steps // chunk
    n_gran = chunk // granule
    gcols = granule * NS            # 512 columns per granule
    Tp = T_steps + chunk            # padded time length (prefill overrun)

    nc = bacc.Bacc()

    hT = nc.declare_dram_parameter("hT", [D, BL, T_steps], F32, isOutput=False)
    WiT = nc.declare_dram_parameter("WiT", [D, DE], F32, isOutput=False)
    EKTs = nc.declare_dram_parameter("EKTs", [D, NE], F32, isOutput=False)
    WihT = nc.declare_dram_parameter("WihT", [DE, 3 * DE], F32, isOutput=False)
    WhhT = nc.declare_dram_parameter("WhhT", [DE, 3 * DE], F32, isOutput=False)
    bias_r = nc.declare_dram_parameter("bias_r", [DE, 1], F32, isOutput=False)
    bias_z = nc.declare_dram_parameter("bias_z", [DE, 1], F32, isOutput=False)
    bias_nh = nc.declare_dram_parameter("bias_nh", [DE, 1], F32, isOutput=False)
    bias_ni = nc.declare_dram_parameter("bias_ni", [DE, 1], F32, isOutput=False)
    s0T = nc.declare_dram_parameter("s0T", [DE, NS], F32, isOutput=False)
    outT = nc.declare_dram_parameter("outT", [DE, BL, T_steps, NE], F32, isOutput=True)

    # staging for w in (t,b,n) order; padded for the last iteration's prefill
    w_stage = nc.dram_tensor("w_stage", [Tp, NS], F32)

    KT = D // 128                   # 8 k-tiles
    ph_cols = min(512, T_steps)     # phase-1 chunk width (within one b)
    n_tb_chunks = (BL * T_steps) // ph_cols
    n_m = ph_cols // 128

    with tile.TileContext(nc) as tc, ExitStack() as ctx:
        const = ctx.enter_context(tc.tile_pool(name="const", bufs=1))

        WiT_sb = const.tile([128, KT, DE], F32)
        nc.sync.dma_start(out=WiT_sb, in_=WiT.rearrange("(k p) d -> p k d", p=128))
        EK_sb = const.tile([128, KT, NE], F32)
        nc.sync.dma_start(out=EK_sb, in_=EKTs.rearrange("(k p) d -> p k d", p=128))
        Whh_sb = const.tile([DE, 3 * DE], F32)
        nc.sync.dma_start(out=Whh_sb, in_=WhhT[:, :])
        Wih_sb = const.tile([DE, 3 * DE], F32)
        nc.sync.dma_start(out=Wih_sb, in_=WihT[:, :])
        br_sb = const.tile([DE, 1], F32)
        nc.sync.dma_start(out=br_sb, in_=bias_r[:, :])
        bz_sb = const.tile([DE, 1], F32)
        nc.sync.dma_start(out=bz_sb, in_=bias_z[:, :])
        bnh_sb = const.tile([DE, 1], F32)
        nc.sync.dma_start(out=bnh_sb, in_=bias_nh[:, :])
        bni_sb = const.tile([DE, 1], F32)
        nc.sync.dma_start(out=bni_sb, in_=bias_ni[:, :])
        s0_sb = const.tile([DE, NS], F32)
        nc.sync.dma_start(out=s0_sb, in_=s0T[:, :])
        ones_sb = const.tile([1, DE], F32)
        nc.vector.memset(ones_sb, 1.0)

        # persistent scan buffers
        hpT_sb = const.tile([DE, BL, Tp], F32)           # (64, b, t) padded
        state_hand = const.tile([DE, NS], F32)
        out_sb = const.tile([DE, chunk * NS], F32)       # col = tl*16 + b*8 + n
        gxw_n_sb = const.tile([DE, chunk * NS], F32)

        # zero-init the pads read by the (harmless) last-iteration prefill
        assert chunk <= 128
        zpad = const.tile([chunk, NS], F32)
        nc.vector.memset(zpad, 0.0)
        nc.sync.dma_start(out=w_stage[T_steps:Tp, :], in_=zpad)
        for b in range(BL):
            nc.vector.memset(hpT_sb[:, b, T_steps:Tp], 0.0)

        # ---------------- phase 1 ----------------
        with tc.tile_pool(name="ph1", bufs=3) as ph1, \
             tc.tile_pool(name="ph1ps", bufs=2, space="PSUM") as ph1ps:
            for c8 in range(n_tb_chunks):
                b_idx = (c8 * ph_cols) // T_steps
                t0 = (c8 * ph_cols) % T_steps
                h_sb = ph1.tile([128, KT, ph_cols], F32, tag="h_sb")
                nc.sync.dma_start(
                    out=h_sb,
                    in_=hT.rearrange("(k p) b t -> p k b t", p=128)[
                        :, :, b_idx, t0:t0 + ph_cols
                    ],
                )
                hp_ps = ph1ps.tile([DE, ph_cols], F32, tag="hp_ps")
                for k in range(KT):
                    nc.tensor.matmul(
                        hp_ps, WiT_sb[:, k, :], h_sb[:, k, :],
                        start=(k == 0), stop=(k == KT - 1),
                    )
                nc.vector.tensor_copy(hpT_sb[:, b_idx, t0:t0 + ph_cols], hp_ps)
                for m in range(n_m):
                    lg_ps = ph1ps.tile([128, NE], F32, tag="lg_ps")
                    for k in range(KT):
                        nc.tensor.matmul(
                            lg_ps,
                            h_sb[:, k, m * 128:(m + 1) * 128],
                            EK_sb[:, k, :],
                            start=(k == 0), stop=(k == KT - 1),
                        )
                    e_sb = ph1.tile([128, NE], F32, tag="e_sb")
                    nc.scalar.activation(e_sb, lg_ps, AF.Exp)
                    ssum = ph1.tile([128, 1], F32, tag="ssum")
                    nc.vector.tensor_reduce(ssum, e_sb, mybir.AxisListType.X, OP.add)
                    rec = ph1.tile([128, 1], F32, tag="rec")
                    nc.vector.reciprocal(rec, ssum)
                    w_sb = ph1.tile([128, NE], F32, tag="w_sb")
                    nc.vector.tensor_scalar_mul(w_sb, e_sb, rec)
                    # w_stage[(t0+m*128+p), b*8+j] = w_sb[p, j]
                    nc.gpsimd.dma_start(
                        out=w_stage[t0 + m * 128: t0 + m * 128 + 128,
                                    b_idx * NE:(b_idx + 1) * NE],
                        in_=w_sb,
                    )

        # ---------------- phase 2: scan ----------------
        spool = ctx.enter_context(tc.tile_pool(name="scan", bufs=4))
        wpool = ctx.enter_context(tc.tile_pool(name="wslab", bufs=2))
        pp = ctx.enter_context(tc.tile_pool(name="scanps", bufs=1, space="PSUM"))

        ps_r = [pp.tile([DE, gcols], F32, tag=f"ps_r{g}", name=f"ps_r{g}")
                for g in range(n_gran)]
        ps_z = [pp.tile([DE, gcols], F32, tag=f"ps_z{g}", name=f"ps_z{g}")
                for g in range(n_gran)]
        ps_p = pp.tile([DE, gcols], F32, tag="ps_p")     # granule slots of 16 cols
        ps_wb = pp.tile([DE, gcols], F32, tag="ps_wb")
        ps_gn = pp.tile([DE, gcols], F32, tag="ps_gn")

        nc.vector.tensor_copy(state_hand, s0_sb)

        def prefill(gl, step_off):
            """Fill ps_rz[gl] (gx_rz) and gxw_n slot gl for steps
            [step_off, step_off+granule). step_off: int or register expr."""
            w_slab = wpool.tile([1, gcols], F32, tag="w_slab")
            if isinstance(step_off, int):
                wsrc = w_stage[step_off:step_off + granule, :]
            else:
                wsrc = w_stage[bass.DynSlice(step_off, granule), :]
            nc.gpsimd.dma_start(out=w_slab, in_=wsrc)
            # broadcast w across 64 partitions via K=1 matmul
            nc.tensor.matmul(ps_wb, ones_sb, w_slab, start=True, stop=True)
            # inpT[d, (t,b,n)] = hpT[d, b, t0+t] * w[(t,b,n)]
            if isinstance(step_off, int):
                base = hpT_sb[:, 0, step_off:step_off + granule]
            else:
                base = hpT_sb[:, 0, bass.DynSlice(step_off, granule)]
            hp_ap = _expand(base, [[1, granule], [Tp, BL], [0, NE]])
            inpT = spool.tile([DE, gcols], F32, tag="inpT")
            nc.vector.tensor_tensor(inpT, hp_ap, ps_wb, OP.mult)
            nc.tensor.matmul(ps_r[gl], Wih_sb[:, 0:DE], inpT,
                             start=True, stop=False, skip_group_check=True)
            nc.tensor.matmul(ps_z[gl], Wih_sb[:, DE:2 * DE], inpT,
                             start=True, stop=False, skip_group_check=True)
            nc.tensor.matmul(ps_gn, Wih_sb[:, 2 * DE:], inpT, start=True, stop=True)
            nc.vector.tensor_copy(gxw_n_sb[:, gl * gcols:(gl + 1) * gcols], ps_gn)

        def scan_step(gl, sl, t_loc):
            cols = slice(t_loc * NS, (t_loc + 1) * NS)
            gc = slice(sl * NS, (sl + 1) * NS)
            s_prev = state_hand if t_loc == 0 else out_sb[:, (t_loc - 1) * NS:t_loc * NS]
            nc.tensor.matmul(ps_r[gl][:, gc], Whh_sb[:, 0:DE], s_prev,
                             start=False, stop=True, skip_group_check=True)
            nc.tensor.matmul(ps_p[:, gc], Whh_sb[:, 2 * DE:], s_prev,
                             start=True, stop=True)
            nc.tensor.matmul(ps_z[gl][:, gc], Whh_sb[:, DE:2 * DE], s_prev,
                             start=False, stop=True, skip_group_check=True)
            r_sb = spool.tile([DE, NS], F32, tag="r_sb")
            nc.scalar.activation(r_sb, ps_r[gl][:, gc], AF.Sigmoid, bias=br_sb)
            v = spool.tile([DE, NS], F32, tag="v")
            nc.vector.scalar_tensor_tensor(v, ps_p[:, gc], bnh_sb, r_sb,
                                           OP.add, OP.mult)
            z_sb = spool.tile([DE, NS], F32, tag="z_sb")
            nc.scalar.activation(z_sb, ps_z[gl][:, gc], AF.Sigmoid, bias=bz_sb)
            u = spool.tile([DE, NS], F32, tag="u")
            nc.vector.tensor_add(u, v, gxw_n_sb[:, cols])
            # off critical path: a = z*s ; oz = 1-z
            a = spool.tile([DE, NS], F32, tag="a")
            nc.vector.tensor_tensor(a, z_sb, s_prev, OP.mult)
            oz = spool.tile([DE, NS], F32, tag="oz")
            nc.vector.tensor_scalar(oz, z_sb, -1.0, 1.0, OP.mult, OP.add)
            n_ = spool.tile([DE, NS], F32, tag="n_")
            nc.scalar.activation(n_, u, AF.Tanh, bias=bni_sb)
            b2 = spool.tile([DE, NS], F32, tag="b2")
            nc.vector.tensor_tensor(b2, oz, n_, OP.mult)
            nc.vector.tensor_tensor(out_sb[:, cols], b2, a, OP.add)

        def body(i_next, i_out):
            """One chunk: scan all granules, prefill the next chunk's
            granules, hand off state, DMA the chunk out. i_next/i_out are
            ints or register exprs."""
            for gl in range(n_gran):
                for sl in range(granule):
                    scan_step(gl, sl, gl * granule + sl)
                prefill(gl, i_next * chunk + gl * granule
                        if isinstance(i_next, int) else
                        i_next * chunk + gl * granule)
            nc.vector.tensor_copy(state_hand, out_sb[:, (chunk - 1) * NS:chunk * NS])
            o4 = outT[:, :, :, :]
            for b in range(BL):
                src = _expand(out_sb[:, b * NE:b * NE + 1],
                              [[NS, chunk], [1, NE]])
                dst = bass.AP(
                    tensor=o4.tensor,
                    offset=i_out * (chunk * NE) + b * (T_steps * NE),
                    ap=[[BL * T_steps * NE, DE], [NE, chunk], [1, NE]],
                )
                nc.sync.dma_start(out=dst, in_=src)

        # prologue: prefill iteration 0
        for gl in range(n_gran):
            prefill(gl, gl * granule)

        if n_iters == 1:
            body(1, 0)
        else:
            with tc.For_i(0, n_iters, 1,
                          hint_engines=(mybir.EngineType.PE,
                                        mybir.EngineType.DVE,
                                        mybir.EngineType.Activation)) as i:
                body(i + 1, i)

    nc.compile()
    return nc


_PROG_CACHE = {}


def _get_prog(T_steps=T, chunk=64):
    key = (T_steps, chunk)
    if key not in _PROG_CACHE:
        _PROG_CACHE[key] = build_program(T_steps, chunk)
    return _PROG_CACHE[key]


def make_in_maps(h_seq, entity_keys, Wi, bi, W_ih, W_hh, b_ih, b_hh, e0):
    common = {
        "WiT": np.ascontiguousarray(Wi.T),
        "EKTs": np.ascontiguousarray((entity_keys / np.sqrt(np.float32(D))).T),
        "WihT": np.ascontiguousarray(W_ih.T),
        "WhhT": np.ascontiguousarray(W_hh.T),
        "bias_r": np.ascontiguousarray((b_ih[:DE] + b_hh[:DE])[:, None]),
        "bias_z": np.ascontiguousarray((b_ih[DE:2 * DE] + b_hh[DE:2 * DE])[:, None]),
        "bias_nh": np.ascontiguousarray(b_hh[2 * DE:][:, None]),
        "bias_ni": np.ascontiguousarray(b_ih[2 * DE:][:, None]),
        "s0T": np.ascontiguousarray(np.tile(e0.T, (1, BL))),
    }
    in_maps = []
    for c in range(NCORES):
        hc = np.ascontiguousarray(
            h_seq[c * BL:(c + 1) * BL].transpose(2, 0, 1))  # (D, BL, T)
        in_maps.append({**common, "hT": hc})
    return in_maps


def kernel(h_seq, entity_keys, Wi, bi, W_ih, W_hh, b_ih, b_hh, e0, _trace=False):
    args = [np.asarray(x, np.float32) for x in
            (h_seq, entity_keys, Wi, bi, W_ih, W_hh, b_ih, b_hh, e0)]
    h_seq = args[0]
    Ts = h_seq.shape[1]
    nc = _get_prog(Ts)
    in_maps = make_in_maps(*args)
    res = run_bass_kernel_spmd(nc, in_maps, list(range(NCORES)), trace=_trace)
    stack = np.empty((B, Ts, NE, DE), np.float32)
    for c in range(NCORES):
        o = res.results[c]["outT"]          # (DE, BL, T, NE)
        stack[c * BL:(c + 1) * BL] = o.transpose(1, 2, 3, 0)
    seq = stack.reshape(B, Ts, NE * DE)
    if _trace:
        return (seq, stack), res
    return seq, stack


# revision 13
# speedup vs baseline: 1.0056x; 1.0056x over previous
"""Trainium2 Bass kernel for nn_EntityTable (GRU entity scan over routed projections).

Sharding: data-parallel over batch B=16 -> 2 batches per core (8 cores).
Host pre-transposes h_seq to (D, b, t) per core and all weights, so the
device never transposes. Phase 1 computes hpT = Wi @ h^T and the routing
softmax (stream-major), staging w in (t,b,n) order. Phase 2 runs the
sequential GRU with gx contributions prefilled into PSUM by PE matmuls so
each step is only: 2 matmuls, 1 sigmoid (r,z fused + folded biases),
1 fused (p+b)*r, 1 add, 1 tanh (folded bias), and a 3-op blend with two
ops off the critical path. Output is written (d,b,t,n) and transposed on
host.
"""

import numpy as np
from contextlib import ExitStack

import concourse.bass as bass
import concourse.bacc as bacc
import concourse.tile as tile
from concourse import mybir
from concourse.bass_utils import run_bass_kernel_spmd

F32 = mybir.dt.float32
AF = mybir.ActivationFunctionType
OP = mybir.AluOpType

B, T, D = 16, 2048, 1024
NE, DE = 8, 64
NCORES = 8
BL = B // NCORES          # 2 batches per core
NS = BL * NE              # 16 streams per core, col = b*8 + n
TBL = BL * T              # 4096 (b,t) columns per core


def _expand(ap, dims):
    """Replace the free dims of a sliced AP with an explicit dim list."""
    return bass.AP(tensor=ap.tensor, offset=ap.offset, ap=[list(ap.ap[0])] + dims)


def build_program(T_steps=T, chunk=128, granule=32):
    """Build the SPMD Bass program (same program on all 8 cores)."""
    assert T_steps % chunk == 0 and chunk % granule == 0
    n_iters = T_steps // chunk
    n_gran = chunk // granule
    gcols = granule * NS            # 512 columns per granule
    Tp = T_steps + chunk            # padded time length (prefill overrun)

    nc = bacc.Bacc()

    hT = nc.declare_dram_parameter("hT", [D, BL, T_steps], F32, isOutput=False)
    WiT = nc.declare_dram_parameter("WiT", [D, DE], F32, isOutput=False)
    EKTs = nc.declare_dram_parameter("EKTs", [D, NE], F32, isOutput=False)
    WihT = nc.declare_dram_parameter("WihT", [DE, 3 * DE], F32, isOutput=False)
    WhhT = nc.declare_dram_parameter("WhhT", [DE, 3 * DE], F32, isOutput=False)
    bias_r = nc.declare_dram_parameter("bias_r", [DE, 1], F32, isOutput=False)
    bias_z = nc.declare_dram_parameter("bias_z", [DE, 1], F32, isOutput=False)
    bias_nh = nc.declare_dram_parameter("bias_nh", [DE, 1], F32, isOutput=False)
    bias_ni = nc.declare_dram_parameter("bias_ni", [DE, 1], F32, isOutput=False)
    s0T = nc.declare_dram_parameter("s0T", [DE, NS], F32, isOutput=False)
    outT = nc.declare_dram_parameter("outT", [DE, BL, T_steps, NE], F32, isOutput=True)

    # staging for w in (t,b,n) order; padded for the last iteration's prefill
    w_stage = nc.dram_tensor("w_stage", [Tp, NS], F32)

    KT = D // 128                   # 8 k-tiles
    ph_cols = min(512, T_steps)     # phase-1 chunk width (within one b)
    n_tb_chunks = (BL * T_steps) // ph_cols
    n_m = ph_cols // 128

    with tile.TileContext(nc) as tc, ExitStack() as ctx:
        const = ctx.enter_context(tc.tile_pool(name="const", bufs=1))

        WiT_sb = const.tile([128, KT, DE], F32)
        nc.sync.dma_start(out=WiT_sb, in_=WiT.rearrange("(k p) d -> p k d", p=128))
        EK_sb = const.tile([128, KT, NE], F32)
        nc.sync.dma_start(out=EK_sb, in_=EKTs.rearrange("(k p) d -> p k d", p=128))
        Whh_sb = const.tile([DE, 3 * DE], F32)
        nc.sync.dma_start(out=Whh_sb, in_=WhhT[:, :])
        Wih_sb = const.tile([DE, 3 * DE], F32)
        nc.sync.dma_start(out=Wih_sb, in_=WihT[:, :])
        br_sb = const.tile([DE, 1], F32)
        nc.sync.dma_start(out=br_sb, in_=bias_r[:, :])
        bz_sb = const.tile([DE, 1], F32)
        nc.sync.dma_start(out=bz_sb, in_=bias_z[:, :])
        bnh_sb = const.tile([DE, 1], F32)
        nc.sync.dma_start(out=bnh_sb, in_=bias_nh[:, :])
        bni_sb = const.tile([DE, 1], F32)
        nc.sync.dma_start(out=bni_sb, in_=bias_ni[:, :])
        s0_sb = const.tile([DE, NS], F32)
        nc.sync.dma_start(out=s0_sb, in_=s0T[:, :])
        ones_sb = const.tile([1, DE], F32)
        nc.vector.memset(ones_sb, 1.0)

        # persistent scan buffers
        hpT_sb = const.tile([DE, BL, Tp], F32)           # (64, b, t) padded
        state_hand = const.tile([DE, NS], F32)
        out_sb = const.tile([DE, chunk * NS], F32)       # col = tl*16 + b*8 + n
        gxw_n_sb = const.tile([DE, chunk * NS], F32)

        # zero-init the pads read by the (harmless) last-iteration prefill
        assert chunk <= 128
        zpad = const.tile([chunk, NS], F32)
        nc.vector.memset(zpad, 0.0)
        nc.sync.dma_start(out=w_stage[T_steps:Tp, :], in_=zpad)
        for b in range(BL):
            nc.vector.memset(hpT_sb[:, b, T_steps:Tp], 0.0)

        # ---------------- phase 1 ----------------
        with tc.tile_pool(name="ph1", bufs=3) as ph1, \
             tc.tile_pool(name="ph1ps", bufs=2, space="PSUM") as ph1ps:
            for c8 in range(n_tb_chunks):
                b_idx = (c8 * ph_cols) // T_steps
                t0 = (c8 * ph_cols) % T_steps
                h_sb = ph1.tile([128, KT, ph_cols], F32, tag="h_sb")
                nc.sync.dma_start(
                    out=h_sb,
                    in_=hT.rearrange("(k p) b t -> p k b t", p=128)[
                        :, :, b_idx, t0:t0 + ph_cols
                    ],
                )
                hp_ps = ph1ps.tile([DE, ph_cols], F32, tag="hp_ps")
                for k in range(KT):
                    nc.tensor.matmul(
                        hp_ps, WiT_sb[:, k, :], h_sb[:, k, :],
                        start=(k == 0), stop=(k == KT - 1),
                    )
                nc.vector.tensor_copy(hpT_sb[:, b_idx, t0:t0 + ph_cols], hp_ps)
                for m in range(n_m):
                    lg_ps = ph1ps.tile([128, NE], F32, tag="lg_ps")
                    for k in range(KT):
                        nc.tensor.matmul(
                            lg_ps,
                            h_sb[:, k, m * 128:(m + 1) * 128],
                            EK_sb[:, k, :],
                            start=(k == 0), stop=(k == KT - 1),
                        )
                    e_sb = ph1.tile([128, NE], F32, tag="e_sb")
                    nc.scalar.activation(e_sb, lg_ps, AF.Exp)
                    ssum = ph1.tile([128, 1], F32, tag="ssum")
                    nc.vector.tensor_reduce(ssum, e_sb, mybir.AxisListType.X, OP.add)
                    rec = ph1.tile([128, 1], F32, tag="rec")
                    nc.vector.reciprocal(rec, ssum)
                    w_sb = ph1.tile([128, NE], F32, tag="w_sb")
                    nc.vector.tensor_scalar_mul(w_sb, e_sb, rec)
                    # w_stage[(t0+m*128+p), b*8+j] = w_sb[p, j]
                    nc.gpsimd.dma_start(
                        out=w_stage[t0 + m * 128: t0 + m * 128 + 128,
                                    b_idx * NE:(b_idx + 1) * NE],
                        in_=w_sb,
                    )

        # ---------------- phase 2: scan ----------------
        spool = ctx.enter_context(tc.tile_pool(name="scan", bufs=12))
        ipool = ctx.enter_context(tc.tile_pool(name="inp", bufs=3))
        wpool = ctx.enter_context(tc.tile_pool(name="wslab", bufs=2))
        pp = ctx.enter_context(tc.tile_pool(name="scanps", bufs=1, space="PSUM"))

        ps_rz = [pp.tile([2 * DE, gcols], F32, tag=f"ps_rz{g}", name=f"ps_rz{g}")
                 for g in range(n_gran)]
        ps_p = pp.tile([DE, gcols], F32, tag="ps_p")     # granule slots of 16 cols
        ps_wb = pp.tile([DE, gcols], F32, tag="ps_wb")
        ps_gn = pp.tile([DE, gcols], F32, tag="ps_gn")

        nc.vector.tensor_copy(state_hand, s0_sb)
        warm = const.tile([DE, 1], F32)
        nc.scalar.activation(warm, s0_sb[:, 0:1], AF.Sigmoid)
        nc.scalar.activation(warm, s0_sb[:, 0:1], AF.Tanh)

        def prefill(gl, step_off):
            """Fill ps_rz[gl] (gx_rz) and gxw_n slot gl for steps
            [step_off, step_off+granule). step_off: int or register expr."""
            w_slab = wpool.tile([1, gcols], F32, tag="w_slab")
            if isinstance(step_off, int):
                wsrc = w_stage[step_off:step_off + granule, :]
            else:
                wsrc = w_stage[bass.DynSlice(step_off, granule), :]
            nc.gpsimd.dma_start(out=w_slab, in_=wsrc)
            # broadcast w across 64 partitions via K=1 matmul
            nc.tensor.matmul(ps_wb, ones_sb, w_slab, start=True, stop=True)
            # inpT[d, (t,b,n)] = hpT[d, b, t0+t] * w[(t,b,n)]
            if isinstance(step_off, int):
                base = hpT_sb[:, 0, step_off:step_off + granule]
            else:
                base = hpT_sb[:, 0, bass.DynSlice(step_off, granule)]
            hp_ap = _expand(base, [[1, granule], [Tp, BL], [0, NE]])
            inpT = ipool.tile([DE, gcols], F32, tag="inpT")
            nc.vector.tensor_tensor(inpT, hp_ap, ps_wb, OP.mult)
            nc.tensor.matmul(ps_rz[gl], Wih_sb[:, 0:2 * DE], inpT,
                             start=True, stop=False, skip_group_check=True)
            nc.tensor.matmul(ps_gn, Wih_sb[:, 2 * DE:], inpT, start=True, stop=True)
            nc.vector.tensor_copy(gxw_n_sb[:, gl * gcols:(gl + 1) * gcols], ps_gn)

        def scan_step(gl, sl, t_loc):
            cols = slice(t_loc * NS, (t_loc + 1) * NS)
            gc = slice(sl * NS, (sl + 1) * NS)
            s_prev = state_hand if t_loc == 0 else out_sb[:, (t_loc - 1) * NS:t_loc * NS]
            nc.tensor.matmul(ps_rz[gl][:, gc], Whh_sb[:, 0:2 * DE], s_prev,
                             start=False, stop=True, skip_group_check=True)
            nc.tensor.matmul(ps_p[:, gc], Whh_sb[:, 2 * DE:], s_prev,
                             start=True, stop=True)
            r_sb = spool.tile([DE, NS], F32, tag="r_sb")
            nc.scalar.activation(r_sb, ps_rz[gl][0:DE, gc], AF.Sigmoid, bias=br_sb)
            v = spool.tile([DE, NS], F32, tag="v")
            nc.vector.scalar_tensor_tensor(v, ps_p[:, gc], bnh_sb, r_sb,
                                           OP.add, OP.mult)
            z_sb = spool.tile([DE, NS], F32, tag="z_sb")
            nc.scalar.activation(z_sb, ps_rz[gl][DE:2 * DE, gc], AF.Sigmoid, bias=bz_sb)
            u = spool.tile([DE, NS], F32, tag="u")
            nc.vector.tensor_add(u, v, gxw_n_sb[:, cols])
            # off critical path: a = z*s ; oz = 1-z
            a = spool.tile([DE, NS], F32, tag="a")
            nc.vector.tensor_tensor(a, z_sb, s_prev, OP.mult)
            oz = spool.tile([DE, NS], F32, tag="oz")
            nc.vector.tensor_scalar(oz, z_sb, -1.0, 1.0, OP.mult, OP.add)
            n_ = spool.tile([DE, NS], F32, tag="n_")
            nc.scalar.activation(n_, u, AF.Tanh, bias=bni_sb)
            b2 = spool.tile([DE, NS], F32, tag="b2")
            nc.vector.tensor_tensor(b2, oz, n_, OP.mult)
            nc.vector.tensor_tensor(out_sb[:, cols], b2, a, OP.add)

        def body(i_next, i_out):
            """One chunk: scan all granules, prefill the next chunk's
            granules, hand off state, DMA the chunk out. i_next/i_out are
            ints or register exprs."""
            for gl in range(n_gran):
                for sl in range(granule):
                    scan_step(gl, sl, gl * granule + sl)
                prefill(gl, i_next * chunk + gl * granule
                        if isinstance(i_next, int) else
                        i_next * chunk + gl * granule)
            nc.vector.tensor_copy(state_hand, out_sb[:, (chunk - 1) * NS:chunk * NS])
            o4 = outT[:, :, :, :]
            for b in range(BL):
                src = _expand(out_sb[:, b * NE:b * NE + 1],
                              [[NS, chunk], [1, NE]])
                dst = bass.AP(
                    tensor=o4.tensor,
                    offset=i_out * (chunk * NE) + b * (T_steps * NE),
                    ap=[[BL * T_steps * NE, DE], [NE, chunk], [1, NE]],
                )
                nc.sync.dma_start(out=dst, in_=src)

        # prologue: prefill iteration 0
        for gl in range(n_gran):
            prefill(gl, gl * granule)

        if n_iters == 1:
            body(1, 0)
        else:
            with tc.For_i(0, n_iters, 1,
                          hint_engines=(mybir.EngineType.PE,
                                        mybir.EngineType.DVE,
                                        mybir.EngineType.Activation)) as i:
                body(i + 1, i)

    nc.compile()
    return nc


_PROG_CACHE = {}


def _get_prog(T_steps=T, chunk=128):
    key = (T_steps, chunk)
    if key not in _PROG_CACHE:
        _PROG_CACHE[key] = build_program(T_steps, chunk)
    return _PROG_CACHE[key]


def make_in_maps(h_seq, entity_keys, Wi, bi, W_ih, W_hh, b_ih, b_hh, e0):
    common = {
        "WiT": np.ascontiguousarray(Wi.T),
        "EKTs": np.ascontiguousarray((entity_keys / np.sqrt(np.float32(D))).T),
        "WihT": np.ascontiguousarray(W_ih.T),
        "WhhT": np.ascontiguousarray(W_hh.T),
        "bias_r": np.ascontiguousarray((b_ih[:DE] + b_hh[:DE])[:, None]),
        "bias_z": np.ascontiguousarray((b_ih[DE:2 * DE] + b_hh[DE:2 * DE])[:, None]),
        "bias_nh": np.ascontiguousarray(b_hh[2 * DE:][:, None]),
        "bias_ni": np.ascontiguousarray(b_ih[2 * DE:][:, None]),
        "s0T": np.ascontiguousarray(np.tile(e0.T, (1, BL))),
    }
    in_maps = []
    for c in range(NCORES):
        hc = np.ascontiguousarray(
            h_seq[c * BL:(c + 1) * BL].transpose(2, 0, 1))  # (D, BL, T)
        in_maps.append({**common, "hT": hc})
    return in_maps


def kernel(h_seq, entity_keys, Wi, bi, W_ih, W_hh, b_ih, b_hh, e0, _trace=False):
    args = [np.asarray(x, np.float32) for x in
            (h_seq, entity_keys, Wi, bi, W_ih, W_hh, b_ih, b_hh, e0)]
    h_seq = args[0]
    Ts = h_seq.shape[1]
    nc = _get_prog(Ts)
    in_maps = make_in_maps(*args)
    res = run_bass_kernel_spmd(nc, in_maps, list(range(NCORES)), trace=_trace)
    stack = np.empty((B, Ts, NE, DE), np.float32)
    for c in range(NCORES):
        o = res.results[c]["outT"]          # (DE, BL, T, NE)
        stack[c * BL:(c + 1) * BL] = o.transpose(1, 2, 3, 0)
    seq = stack.reshape(B, Ts, NE * DE)
    if _trace:
        return (seq, stack), res
    return seq, stack


# revision 15
# speedup vs baseline: 1.1374x; 1.1310x over previous
"""Trainium2 Bass kernel for nn_EntityTable (GRU entity scan over routed projections).

Sharding: data-parallel over batch B=16 -> 2 batches per core (8 cores).
Host pre-transposes h_seq to (D, b, t) per core and all weights, so the
device never transposes. Phase 1 computes hpT = Wi @ h^T and the routing
softmax (stream-major), staging w in (t,b,n) order. Phase 2 runs the
sequential GRU with gx contributions prefilled into PSUM by PE matmuls so
each step is only: 2 matmuls, 1 sigmoid (r,z fused + folded biases),
1 fused (p+b)*r, 1 add, 1 tanh (folded bias), and a 3-op blend with two
ops off the critical path. Output is written (d,b,t,n) and transposed on
host.
"""

import numpy as np
from contextlib import ExitStack

import concourse.bass as bass
import concourse.bacc as bacc
import concourse.tile as tile
from concourse import mybir
from concourse.bass_utils import run_bass_kernel_spmd

F32 = mybir.dt.float32
F32R = mybir.dt.float32r
AF = mybir.ActivationFunctionType
OP = mybir.AluOpType

B, T, D = 16, 2048, 1024
NE, DE = 8, 64
NCORES = 8
BL = B // NCORES          # 2 batches per core
NS = BL * NE              # 16 streams per core, col = b*8 + n
TBL = BL * T              # 4096 (b,t) columns per core


def _expand(ap, dims):
    """Replace the free dims of a sliced AP with an explicit dim list."""
    return bass.AP(tensor=ap.tensor, offset=ap.offset, ap=[list(ap.ap[0])] + dims)


def build_program(T_steps=T, chunk=128, granule=32):
    """Build the SPMD Bass program (same program on all 8 cores)."""
    assert T_steps % chunk == 0 and chunk % granule == 0
    n_iters = T_steps // chunk
    n_gran = chunk // granule
    gcols = granule * NS            # 512 columns per granule
    Tp = T_steps + chunk            # padded time length (prefill overrun)

    nc = bacc.Bacc()

    hT = nc.declare_dram_parameter("hT", [D, BL, T_steps], F32, isOutput=False)
    WiT = nc.declare_dram_parameter("WiT", [D, DE], F32, isOutput=False)
    EKTs = nc.declare_dram_parameter("EKTs", [D, NE], F32, isOutput=False)
    WihT = nc.declare_dram_parameter("WihT", [DE, 3 * DE], F32, isOutput=False)
    WhhT = nc.declare_dram_parameter("WhhT", [DE, 3 * DE], F32, isOutput=False)
    bias_r = nc.declare_dram_parameter("bias_r", [DE, 1], F32, isOutput=False)
    bias_z = nc.declare_dram_parameter("bias_z", [DE, 1], F32, isOutput=False)
    bias_nh = nc.declare_dram_parameter("bias_nh", [DE, 1], F32, isOutput=False)
    bias_ni = nc.declare_dram_parameter("bias_ni", [DE, 1], F32, isOutput=False)
    s0T = nc.declare_dram_parameter("s0T", [DE, NS], F32, isOutput=False)
    outT = nc.declare_dram_parameter("outT", [DE, BL, T_steps, NE], F32, isOutput=True)

    # staging for w in (t,b,n) order; padded for the last iteration's prefill
    w_stage = nc.dram_tensor("w_stage", [Tp, NS], F32)

    KT = D // 128                   # 8 k-tiles
    ph_cols = min(512, T_steps)     # phase-1 chunk width (within one b)
    n_tb_chunks = (BL * T_steps) // ph_cols
    n_m = ph_cols // 128

    with tile.TileContext(nc) as tc, ExitStack() as ctx:
        const = ctx.enter_context(tc.tile_pool(name="const", bufs=1))

        WiT_sb = const.tile([128, KT, DE], F32)
        nc.sync.dma_start(out=WiT_sb, in_=WiT.rearrange("(k p) d -> p k d", p=128))
        EK_sb = const.tile([128, KT, NE], F32)
        nc.sync.dma_start(out=EK_sb, in_=EKTs.rearrange("(k p) d -> p k d", p=128))
        Whh_sb = const.tile([DE, 3 * DE], F32)
        nc.sync.dma_start(out=Whh_sb.bitcast(F32R), in_=WhhT[:, :].bitcast(F32R))
        Wih_sb = const.tile([DE, 3 * DE], F32)
        nc.sync.dma_start(out=Wih_sb, in_=WihT[:, :])
        br_sb = const.tile([DE, 1], F32)
        nc.sync.dma_start(out=br_sb, in_=bias_r[:, :])
        bz_sb = const.tile([DE, 1], F32)
        nc.sync.dma_start(out=bz_sb, in_=bias_z[:, :])
        bnh_row = const.tile([1, DE], F32)
        nc.sync.dma_start(out=bnh_row, in_=bias_nh[:, 0:1])
        bni_sb = const.tile([DE, 1], F32)
        nc.sync.dma_start(out=bni_sb, in_=bias_ni[:, :])
        s0_sb = const.tile([DE, NS], F32)
        nc.sync.dma_start(out=s0_sb, in_=s0T[:, :])
        ones_sb = const.tile([1, DE], F32)
        nc.vector.memset(ones_sb, 1.0)
        ones_g = const.tile([1, 32 * NS], F32)
        nc.vector.memset(ones_g, 1.0)

        # persistent scan buffers
        hpT_sb = const.tile([DE, BL, Tp], F32)           # (64, b, t) padded
        state_hand = const.tile([DE, NS], F32)
        out_sb = const.tile([DE, chunk * NS], F32)       # col = tl*16 + b*8 + n
        gxw_n_sb = const.tile([DE, chunk * NS], F32)

        # zero-init the pads read by the (harmless) last-iteration prefill
        assert chunk <= 128
        zpad = const.tile([chunk, NS], F32)
        nc.vector.memset(zpad, 0.0)
        nc.sync.dma_start(out=w_stage[T_steps:Tp, :], in_=zpad)
        for b in range(BL):
            nc.vector.memset(hpT_sb[:, b, T_steps:Tp], 0.0)

        # ---------------- phase 1 ----------------
        with tc.tile_pool(name="ph1", bufs=3) as ph1, \
             tc.tile_pool(name="ph1ps", bufs=2, space="PSUM") as ph1ps:
            for c8 in range(n_tb_chunks):
                b_idx = (c8 * ph_cols) // T_steps
                t0 = (c8 * ph_cols) % T_steps
                h_sb = ph1.tile([128, KT, ph_cols], F32, tag="h_sb")
                nc.sync.dma_start(
                    out=h_sb,
                    in_=hT.rearrange("(k p) b t -> p k b t", p=128)[
                        :, :, b_idx, t0:t0 + ph_cols
                    ],
                )
                hp_ps = ph1ps.tile([DE, ph_cols], F32, tag="hp_ps")
                for k in range(KT):
                    nc.tensor.matmul(
                        hp_ps, WiT_sb[:, k, :], h_sb[:, k, :],
                        start=(k == 0), stop=(k == KT - 1),
                    )
                nc.vector.tensor_copy(hpT_sb[:, b_idx, t0:t0 + ph_cols], hp_ps)
                for m in range(n_m):
                    lg_ps = ph1ps.tile([128, NE], F32, tag="lg_ps")
                    for k in range(KT):
                        nc.tensor.matmul(
                            lg_ps,
                            h_sb[:, k, m * 128:(m + 1) * 128],
                            EK_sb[:, k, :],
                            start=(k == 0), stop=(k == KT - 1),
                        )
                    e_sb = ph1.tile([128, NE], F32, tag="e_sb")
                    nc.scalar.activation(e_sb, lg_ps, AF.Exp)
                    ssum = ph1.tile([128, 1], F32, tag="ssum")
                    nc.vector.tensor_reduce(ssum, e_sb, mybir.AxisListType.X, OP.add)
                    rec = ph1.tile([128, 1], F32, tag="rec")
                    nc.vector.reciprocal(rec, ssum)
                    w_sb = ph1.tile([128, NE], F32, tag="w_sb")
                    nc.vector.tensor_scalar_mul(w_sb, e_sb, rec)
                    # w_stage[(t0+m*128+p), b*8+j] = w_sb[p, j]
                    nc.gpsimd.dma_start(
                        out=w_stage[t0 + m * 128: t0 + m * 128 + 128,
                                    b_idx * NE:(b_idx + 1) * NE],
                        in_=w_sb,
                    )

        # ---------------- phase 2: scan ----------------
        spool = ctx.enter_context(tc.tile_pool(name="scan", bufs=12))
        ipool = ctx.enter_context(tc.tile_pool(name="inp", bufs=3))
        wpool = ctx.enter_context(tc.tile_pool(name="wslab", bufs=2))
        pp = ctx.enter_context(tc.tile_pool(name="scanps", bufs=1, space="PSUM"))

        ps_rz = [pp.tile([2 * DE, gcols], F32, tag=f"ps_rz{g}", name=f"ps_rz{g}")
                 for g in range(n_gran)]
        ps_p = pp.tile([DE, gcols], F32, tag="ps_p")     # granule slots of 16 cols
        ps_wb = pp.tile([DE, gcols], F32, tag="ps_wb")
        ps_gn = pp.tile([DE, gcols], F32, tag="ps_gn")

        nc.vector.tensor_copy(state_hand.bitcast(F32R), s0_sb)
        warm = const.tile([DE, 1], F32)
        nc.scalar.activation(warm, s0_sb[:, 0:1], AF.Sigmoid)
        nc.scalar.activation(warm, s0_sb[:, 0:1], AF.Tanh)

        def prefill(gl, step_off):
            """Fill ps_rz[gl] (gx_rz) and gxw_n slot gl for steps
            [step_off, step_off+granule). step_off: int or register expr."""
            w_slab = wpool.tile([1, gcols], F32, tag="w_slab")
            if isinstance(step_off, int):
                wsrc = w_stage[step_off:step_off + granule, :]
            else:
                wsrc = w_stage[bass.DynSlice(step_off, granule), :]
            nc.gpsimd.dma_start(out=w_slab, in_=wsrc)
            # broadcast w across 64 partitions via K=1 matmul
            nc.tensor.matmul(ps_wb, ones_sb, w_slab, start=True, stop=True)
            # inpT[d, (t,b,n)] = hpT[d, b, t0+t] * w[(t,b,n)]
            if isinstance(step_off, int):
                base = hpT_sb[:, 0, step_off:step_off + granule]
            else:
                base = hpT_sb[:, 0, bass.DynSlice(step_off, granule)]
            hp_ap = _expand(base, [[1, granule], [Tp, BL], [0, NE]])
            inpT = ipool.tile([DE, gcols], F32, tag="inpT")
            nc.vector.tensor_tensor(inpT, hp_ap, ps_wb, OP.mult)
            nc.tensor.matmul(ps_rz[gl], Wih_sb[:, 0:2 * DE], inpT,
                             start=True, stop=False, skip_group_check=True)
            nc.tensor.matmul(ps_gn, Wih_sb[:, 2 * DE:], inpT, start=True, stop=True)
            nc.vector.tensor_copy(gxw_n_sb[:, gl * gcols:(gl + 1) * gcols], ps_gn)

        def scan_step(gl, sl, t_loc):
            cols = slice(t_loc * NS, (t_loc + 1) * NS)
            gc = slice(sl * NS, (sl + 1) * NS)
            s_prev = state_hand if t_loc == 0 else out_sb[:, (t_loc - 1) * NS:t_loc * NS]
            nc.tensor.matmul(ps_rz[gl][:, gc], Whh_sb[:, 0:2 * DE].bitcast(F32R),
                             s_prev.bitcast(F32R),
                             start=False, stop=True, skip_group_check=True)
            nc.tensor.matmul(ps_p[:, gc], Whh_sb[:, 2 * DE:].bitcast(F32R),
                             s_prev.bitcast(F32R),
                             start=False, stop=True, skip_group_check=True)
            r_sb = spool.tile([DE, NS], F32, tag="r_sb")
            nc.scalar.activation(r_sb, ps_rz[gl][0:DE, gc], AF.Sigmoid, bias=br_sb)
            v = spool.tile([DE, NS], F32, tag="v")
            nc.vector.tensor_tensor(v, ps_p[:, gc], r_sb, OP.mult)
            z_sb = spool.tile([DE, NS], F32, tag="z_sb")
            nc.scalar.activation(z_sb, ps_rz[gl][DE:2 * DE, gc], AF.Sigmoid, bias=bz_sb)
            u = spool.tile([DE, NS], F32, tag="u")
            nc.vector.tensor_add(u, v, gxw_n_sb[:, cols])
            # off critical path: a = z*s ; oz = 1-z
            a = spool.tile([DE, NS], F32, tag="a")
            nc.vector.tensor_tensor(a, z_sb, s_prev, OP.mult)
            oz = spool.tile([DE, NS], F32, tag="oz")
            nc.vector.tensor_scalar(oz, z_sb, -1.0, 1.0, OP.mult, OP.add)
            n_ = spool.tile([DE, NS], F32, tag="n_")
            nc.scalar.activation(n_, u, AF.Tanh, bias=bni_sb)
            b2 = spool.tile([DE, NS], F32, tag="b2")
            nc.vector.tensor_tensor(b2, oz, n_, OP.mult)
            nc.vector.tensor_tensor(out_sb[:, cols].bitcast(F32R), b2, a, OP.add)

        def body(i_next, i_out):
            """One chunk: scan all granules, prefill the next chunk's
            granules, hand off state, DMA the chunk out. i_next/i_out are
            ints or register exprs."""
            for gl in range(n_gran):
                nc.tensor.matmul(ps_p, bnh_row, ones_g, start=True, stop=False,
                                 skip_group_check=True)
                for sl in range(granule):
                    scan_step(gl, sl, gl * granule + sl)
                prefill(gl, i_next * chunk + gl * granule
                        if isinstance(i_next, int) else
                        i_next * chunk + gl * granule)
            nc.vector.tensor_copy(state_hand.bitcast(F32R), out_sb[:, (chunk - 1) * NS:chunk * NS])
            o4 = outT[:, :, :, :]
            for b in range(BL):
                src = _expand(out_sb[:, b * NE:b * NE + 1],
                              [[NS, chunk], [1, NE]])
                dst = bass.AP(
                    tensor=o4.tensor,
                    offset=i_out * (chunk * NE) + b * (T_steps * NE),
                    ap=[[BL * T_steps * NE, DE], [NE, chunk], [1, NE]],
                )
                nc.sync.dma_start(out=dst, in_=src)

        # prologue: prefill iteration 0
        for gl in range(n_gran):
            prefill(gl, gl * granule)

        if n_iters == 1:
            body(1, 0)
        else:
            with tc.For_i(0, n_iters, 1,
                          hint_engines=(mybir.EngineType.PE,
                                        mybir.EngineType.DVE,
                                        mybir.EngineType.Activation)) as i:
                body(i + 1, i)

    nc.compile()
    return nc


_PROG_CACHE = {}


def _get_prog(T_steps=T, chunk=128):
    key = (T_steps, chunk)
    if key not in _PROG_CACHE:
        _PROG_CACHE[key] = build_program(T_steps, chunk)
    return _PROG_CACHE[key]


def make_in_maps(h_seq, entity_keys, Wi, bi, W_ih, W_hh, b_ih, b_hh, e0):
    common = {
        "WiT": np.ascontiguousarray(Wi.T),
        "EKTs": np.ascontiguousarray((entity_keys / np.sqrt(np.float32(D))).T),
        "WihT": np.ascontiguousarray(W_ih.T),
        "WhhT": np.ascontiguousarray(W_hh.T),
        "bias_r": np.ascontiguousarray((b_ih[:DE] + b_hh[:DE])[:, None]),
        "bias_z": np.ascontiguousarray((b_ih[DE:2 * DE] + b_hh[DE:2 * DE])[:, None]),
        "bias_nh": np.ascontiguousarray(b_hh[2 * DE:][:, None]),
        "bias_ni": np.ascontiguousarray(b_ih[2 * DE:][:, None]),
        "s0T": np.ascontiguousarray(np.tile(e0.T, (1, BL))),
    }
    in_maps = []
    for c in range(NCORES):
        hc = np.ascontiguousarray(
            h_seq[c * BL:(c + 1) * BL].transpose(2, 0, 1))  # (D, BL, T)
        in_maps.append({**common, "hT": hc})
    return in_maps


def kernel(h_seq, entity_keys, Wi, bi, W_ih, W_hh, b_ih, b_hh, e0, _trace=False):
    args = [np.asarray(x, np.float32) for x in
            (h_seq, entity_keys, Wi, bi, W_ih, W_hh, b_ih, b_hh, e0)]
    h_seq = args[0]
    Ts = h_seq.shape[1]
    nc = _get_prog(Ts)
    in_maps = make_in_maps(*args)
    res = run_bass_kernel_spmd(nc, in_maps, list(range(NCORES)), trace=_trace)
    stack = np.empty((B, Ts, NE, DE), np.float32)
    for c in range(NCORES):
        o = res.results[c]["outT"]          # (DE, BL, T, NE)
        stack[c * BL:(c + 1) * BL] = o.transpose(1, 2, 3, 0)
    seq = stack.reshape(B, Ts, NE * DE)
    if _trace:
        return (seq, stack), res
    return seq, stack
